# revision 22
# baseline (speedup 1.0000x reference)
"""BitLinear forward (RMSNorm -> int8 activation quant -> ternary weight quant
-> matmul -> rescale) on 8 Trainium2 NeuronCores.

Sharding: data-parallel over rows. x (4,4096,1024) flattens to (16384,1024);
each core gets 2048 rows and the full weight (4096,1024).

Key layout/schedule choices (vs the earlier two-pass version):
 - The weight is transposed and block-reordered on the HOST into `wTs` so the
   device can DMA contiguous 2 MiB stage blocks that are already in
   [k-partition, (j, n)] layout -- no PE transposes and no second weight read.
 - w_scale = mean|w| is computed from a per-core 1/8 shard followed by a
   scalar AllReduce, so ternarization starts ~20 us in instead of ~110 us.
 - Ternarize w^T tiles as they stream in (clip in w-units on DVE, +C magic
   rescale on ACT, -C to bf16 alternating DVE/ACT).
 - x path: per row-tile stats (ACT square-accum / GPSIMD abs-max), quantize
   via magic-constant RNE (ACT + GPSIMD), transpose via PE identity matmuls.
 - Main matmul: 4 column blocks x 16 row tiles, each group = 2 PSUM banks,
   k-contiguous 16 matmuls; PSUM -> SBUF copy applies cs = x_scale*w_scale
   and emits fp16 (host upcasts to fp32). Output DMA alternates queues.
 - Emissions are interleaved in expected execution order because per-engine
   instruction streams are FIFO.

Math notes:
 - x_q in [-128,127] and w_t in {-1,0,1} are exact in bf16, so bf16 matmul
   with fp32 PSUM accumulation reproduces the fp32 reference einsum exactly.
 - round-half-to-even via the magic constant 1.5*2^23.
 - ternary quantize sign(ws)*(|ws|>0.5) == RNE(clip(ws,-1,1)), computed as
   RNE(clip(w, -(s+eps), s+eps) * inv) with inv = 1/(s+eps), all in fp32.
 - |out| <= ~3, so fp16 output rounding adds < 2e-4 relative error.
"""

import os

import numpy as np

import concourse.bass as bass
import concourse.mybir as mybir
import concourse.tile as tile
from concourse import bacc
from concourse.bass_utils import run_bass_kernel_spmd
from concourse.masks import make_identity
from concourse import bass_isa

F32 = mybir.dt.float32
F16 = mybir.dt.float16
BF16 = mybir.dt.bfloat16
ALU = mybir.AluOpType
AF = mybir.ActivationFunctionType
AXX = mybir.AxisListType.X

N_CORES = 8
R_FULL, K, N = 16384, 1024, 4096
R = R_FULL // N_CORES          # 2048 rows per core
RT = R // 128                  # 16 row tiles per core
KT = K // 128                  # 8 k-tiles
NCB = 4                        # n column blocks of 1024
CBW = N // NCB                 # 1024

C_MAGIC = 12582912.0           # 1.5 * 2^23: fp32 round-to-nearest-even trick
Q_EPS = 1e-5
NORM_EPS = 1e-6


def build_nc(g_is_ones: bool):
    nc = bacc.Bacc("TRN2", target_bir_lowering=False)

    x_d = nc.dram_tensor("x", [R, K], F32, kind="ExternalInput")
    # wTs row ((cb*2+kh)*128 + p), col (jj*1024 + n) holds
    # weight[cb*1024+n, (kh*4+jj)*128+p]  (see host code below)
    wTs_d = nc.dram_tensor("wTs", [K, N], F32, kind="ExternalInput")
    wsh_d = nc.dram_tensor("wsh", [128, N], F32, kind="ExternalInput")
    cc_in = nc.dram_tensor("cc_in", [1, 1], F32)
    cc_out = nc.dram_tensor("cc_out", [1, 1], F32, addr_space="Shared")
    ccw_in = nc.dram_tensor("ccw_in", [1, 1], F32)
    ccw_out = nc.dram_tensor("ccw_out", [1, 1], F32, addr_space="Shared")
    if not g_is_ones:
        g_d = nc.dram_tensor("g", [1, K], F32, kind="ExternalInput")
    out_d = nc.dram_tensor("out", [R, N], F16, kind="ExternalOutput")

    with tile.TileContext(nc) as tc:
        with (
            tc.tile_pool(name="persist", bufs=1) as persist,
            tc.tile_pool(name="stg", bufs=2) as stg_pool,
            tc.tile_pool(name="wclip", bufs=2) as wclip_pool,
            tc.tile_pool(name="wmag", bufs=2) as wmag_pool,
            tc.tile_pool(name="xt", bufs=6) as xt_pool,
            tc.tile_pool(name="dumb", bufs=2) as dumb_pool,
            tc.tile_pool(name="ux", bufs=3) as ux_pool,
            tc.tile_pool(name="xq", bufs=3) as xq_pool,
            tc.tile_pool(name="xqT", bufs=16) as xqT_pool,
            tc.tile_pool(name="st", bufs=48) as st_pool,
            tc.tile_pool(name="csp", bufs=16) as cs_pool,
            tc.tile_pool(name="osb", bufs=4) as osb_pool,
            tc.tile_pool(name="pmm", bufs=3, space="PSUM") as psum_mm,
            tc.tile_pool(name="ptp", bufs=2, space="PSUM") as psum_tp,
        ):
            # ---- constants (identity is emitted after the scale chain) ----
            ident = persist.tile([128, 128], BF16, tag="ident")
            cbm = persist.tile([128, 1], F32, tag="cbm")
            nc.vector.memset(cbm[:], C_MAGIC)
            ones_f = persist.tile([128, 1], F32, tag="ones_f")
            nc.vector.memset(ones_f[:], 1.0)
            neps = persist.tile([128, 1], F32, tag="neps")
            nc.vector.memset(neps[:], NORM_EPS)

            if not g_is_ones:
                g_row = persist.tile([1, K], F32, tag="g_row")
                nc.sync.dma_start(g_row[:], g_d[:])
                g_b = persist.tile([128, K], F32, tag="g_b")
                nc.gpsimd.partition_broadcast(g_b[:], g_row[0:1, :])

            # persistent ternarized transposed weight: [:, j, n]
            wTT = persist.tile([128, KT, N], BF16, tag="wTT", name="wTT")
            # w_scale broadcast tiles
            wsum_g = persist.tile([128, 1], F32, tag="wsum_g")
            wsb127 = persist.tile([128, 1], F32, tag="wsb127")
            th = persist.tile([128, 1], F32, tag="th")
            nth = persist.tile([128, 1], F32, tag="nth")
            invb = persist.tile([128, 1], F32, tag="invb")

            xqT_tiles = [None] * RT
            cs_tiles = [None] * RT
            srow_tiles = [None] * RT
            axr_tiles = [None] * RT
            stg_tiles = {}

            # ---------------- emitters ----------------

            def emit_wscale_head():
                """shard |w| sum -> AllReduce kick-off (scale chain part 1)"""
                with nc.named_scope("w_scale"):
                    wch = stg_pool.tile([128, N], F32, tag="stg",
                                        name="wsh_stage")
                    nc.sync.dma_start(wch[:], wsh_d[:])
                    wpart = st_pool.tile([128, 1], F32, tag="wpart")
                    # dummy activation output goes into wTT scratch (row j=7
                    # is ternarized last, long after this is consumed)
                    nc.scalar.activation(wTT[:, 7, :], wch[:], AF.Abs,
                                         accum_out=wpart[:])
                    # partition-reduce via PE: ones.T @ wpart -> [1, 1]
                    pred = psum_tp.tile([128, 512], F32, tag="tp",
                                        name="pred")
                    nc.tensor.matmul(pred[0:1, 0:1], lhsT=ones_f[:],
                                     rhs=wpart[:])
                    wsumb = st_pool.tile([1, 1], F32, tag="s1", name="wsumb")
                    nc.vector.tensor_copy(wsumb[:], pred[0:1, 0:1])
                    nc.sync.dma_start(cc_in[:], wsumb[0:1, :])
                    nc.gpsimd.collective_compute(
                        "AllReduce", ALU.add,
                        replica_groups=[list(range(N_CORES))],
                        ins=[cc_in[:]], outs=[cc_out[:]])
                    wsum1 = st_pool.tile([1, 1], F32, tag="s1", name="wsum1")
                    nc.sync.dma_start(wsum1[:], cc_out[:])
                    return wsum1

            def emit_wscale_tail(wsum1):
                """broadcast + derived scalars (scale chain part 2)"""
                with nc.named_scope("w_scale"):
                    nc.gpsimd.partition_broadcast(wsum_g[:], wsum1[0:1, :])
                    # th = mean|w| + eps ; nth = -th ; invb = 1/th
                    nc.vector.tensor_scalar(
                        out=th[:], in0=wsum_g[:], scalar1=1.0 / (N * K),
                        scalar2=Q_EPS, op0=ALU.mult, op1=ALU.add)
                    nc.vector.tensor_scalar(
                        out=nth[:], in0=wsum_g[:], scalar1=-1.0 / (N * K),
                        scalar2=-Q_EPS, op0=ALU.mult, op1=ALU.add)
                    nc.vector.reciprocal(invb[:], th[:])
                    # wsb127 = mean|w| / 127   (for cs = axr * wsb127)
                    nc.vector.tensor_scalar(
                        out=wsb127[:], in0=wsum_g[:],
                        scalar1=1.0 / (N * K) / 127.0,
                        scalar2=None, op0=ALU.mult)

            def emit_stage(cb, kh):
                stg = stg_pool.tile([128, N], F32, tag="stg",
                                    name=f"stg{cb}_{kh}")
                r0 = (cb * 2 + kh) * 128
                nc.sync.dma_start(stg[:], wTs_d[r0:r0 + 128, :])
                stg_tiles[(cb, kh)] = stg

            def emit_tern(cb, j):
                with nc.named_scope("w_ternarize"):
                    stg = stg_tiles[(cb, j // 4)]
                    jj = j % 4
                    src = stg[:, jj * CBW:(jj + 1) * CBW]
                    wc = wclip_pool.tile([128, CBW], F32, tag="wc",
                                         name=f"wc{cb}_{j}")
                    nc.vector.tensor_scalar(
                        out=wc[:], in0=src, scalar1=th[:, 0:1],
                        scalar2=nth[:, 0:1], op0=ALU.min, op1=ALU.max)
                    wm = wmag_pool.tile([128, CBW], F32, tag="wm",
                                        name=f"wm{cb}_{j}")
                    dst = wTT[:, j, cb * CBW:(cb + 1) * CBW]
                    if j % 2 == 0:
                        nc.scalar.activation(wm[:], wc[:], AF.Identity,
                                             bias=cbm[:, 0:1],
                                             scale=invb[:, 0:1])
                        nc.vector.tensor_scalar(
                            out=dst, in0=wm[:], scalar1=C_MAGIC,
                            scalar2=None, op0=ALU.subtract)
                    else:
                        nc.vector.tensor_scalar(
                            out=wm[:], in0=wc[:], scalar1=invb[:, 0:1],
                            scalar2=cbm[:, 0:1], op0=ALU.mult, op1=ALU.add)
                        nc.scalar.activation(dst, wm[:], AF.Copy,
                                             bias=-C_MAGIC)

            def emit_x_front(rt):
                """DMA + stats + srow chain (no ux/xq/transpose yet)"""
                with nc.named_scope("x_quant"):
                    xt = xt_pool.tile([128, K], F32, tag="xt", name=f"xt{rt}")
                    nc.sync.dma_start(xt[:], x_d[rt * 128:(rt + 1) * 128, :])
                    if g_is_ones:
                        xg = xt
                    else:
                        xg = ux_pool.tile([128, K], F32, tag="xg",
                                          name=f"xg{rt}")
                        nc.vector.tensor_mul(xg[:], xt[:], g_b[:])
                    du = dumb_pool.tile([128, K], BF16, tag="dumb",
                                        name=f"xsq{rt}")
                    ssq = st_pool.tile([128, 1], F32, tag="s", name=f"ssq{rt}")
                    nc.scalar.activation(du[:], xt[:], AF.Square,
                                         accum_out=ssq[:])
                    am = st_pool.tile([128, 1], F32, tag="s", name=f"am{rt}")
                    nc.vector.tensor_reduce(am[:], xg[:], axis=AXX,
                                            op=ALU.max,
                                            apply_absolute_value=True)
                    # s0 = sqrt(ssq/K + eps) in one ACT op
                    s0 = st_pool.tile([128, 1], F32, tag="s", name=f"s0{rt}")
                    nc.scalar.activation(s0[:], ssq[:], AF.Sqrt,
                                         bias=neps[:, 0:1], scale=1.0 / K)
                    rs = st_pool.tile([128, 1], F32, tag="s", name=f"rs{rt}")
                    nc.vector.reciprocal(rs[:], s0[:])
                    axr = st_pool.tile([128, 1], F32, tag="s", name=f"axr{rt}")
                    nc.vector.tensor_mul(axr[:], am[:], rs[:])
                    sx = st_pool.tile([128, 1], F32, tag="s", name=f"sx{rt}")
                    nc.vector.tensor_scalar(
                        out=sx[:], in0=axr[:], scalar1=1.0 / 127.0,
                        scalar2=Q_EPS, op0=ALU.mult, op1=ALU.add)
                    dx = st_pool.tile([128, 1], F32, tag="s", name=f"dx{rt}")
                    nc.vector.reciprocal(dx[:], sx[:])
                    srow = st_pool.tile([128, 1], F32, tag="s",
                                        name=f"srow{rt}")
                    nc.vector.tensor_mul(srow[:], rs[:], dx[:])
                    srow_tiles[rt] = srow
                    axr_tiles[rt] = axr
                    return xt if g_is_ones else xg

            def emit_x_cs(rt):
                with nc.named_scope("x_quant"):
                    cs = cs_pool.tile([128, 1], F32, tag="cs", name=f"cs{rt}")
                    nc.vector.tensor_mul(cs[:], axr_tiles[rt][:], wsb127[:])
                    cs_tiles[rt] = cs

            def emit_x_back(rt, xg):
                with nc.named_scope("x_quant"):
                    srow = srow_tiles[rt]
                    ux = ux_pool.tile([128, K], F32, tag="ux", name=f"ux{rt}")
                    nc.vector.tensor_scalar(
                        out=ux[:], in0=xg[:], scalar1=srow[:, 0:1],
                        scalar2=cbm[:, 0:1], op0=ALU.mult, op1=ALU.add)
                    xq = xq_pool.tile([128, K], BF16, tag="xq", name=f"xq{rt}")
                    nc.scalar.activation(xq[:], ux[:], AF.Copy,
                                         bias=-C_MAGIC)
                    xqT = xqT_pool.tile([128, KT, 128], BF16, tag="xqT",
                                        name=f"xqT{rt}")
                    for gq in range(2):
                        tp = psum_tp.tile([128, 512], F32, tag="tp",
                                          name=f"tpx{rt}_{gq}")
                        for jj in range(4):
                            j = gq * 4 + jj
                            nc.tensor.matmul(
                                tp[:, jj * 128:(jj + 1) * 128],
                                lhsT=xq[:, j * 128:(j + 1) * 128],
                                rhs=ident[:])
                        dst = xqT[:, gq * 4:(gq + 1) * 4, :]
                        if gq == 0:
                            nc.vector.tensor_copy(dst, tp[:])
                        else:
                            nc.scalar.copy(dst, tp[:])
                    xqT_tiles[rt] = xqT

            def emit_mm(cb, rt, gi):
                xqT = xqT_tiles[rt]
                cs = cs_tiles[rt]
                c0 = cb * CBW
                with nc.named_scope("mm"):
                    pt = psum_mm.tile([128, CBW], F32, tag="pmm",
                                      name=f"p{cb}_{rt}")
                    for j in range(KT):
                        nc.tensor.matmul(
                            pt[:, 0:512], lhsT=xqT[:, j, :],
                            rhs=wTT[:, j, c0:c0 + 512],
                            start=(j == 0), stop=(j == KT - 1))
                        nc.tensor.matmul(
                            pt[:, 512:CBW], lhsT=xqT[:, j, :],
                            rhs=wTT[:, j, c0 + 512:c0 + CBW],
                            start=(j == 0), stop=(j == KT - 1))
                with nc.named_scope("out_scale"):
                    osb = osb_pool.tile([128, CBW], F16, tag="osb",
                                        name=f"osb{cb}_{rt}")
                    if gi % 2 == 0:
                        nc.scalar.activation(osb[:], pt[:], AF.Copy,
                                             scale=cs[:, 0:1])
                    else:
                        nc.vector.tensor_scalar(
                            out=osb[:], in0=pt[:], scalar1=cs[:, 0:1],
                            scalar2=None, op0=ALU.mult)
                    eng = nc.sync if gi % 2 == 0 else nc.scalar
                    eng.dma_start(
                        out_d[rt * 128:(rt + 1) * 128, c0:c0 + CBW], osb[:])

            # ---------------- emission schedule ----------------
            # Interleaved in expected execution order (engine streams are
            # FIFO).  Scale chain first; x stats pipelined one tile ahead;
            # mm groups in diagonal order d = rt + 6*cb so early matmuls only
            # need column-block 0 while the x pipeline and later ternarize
            # blocks ramp up; x/tern/stage emissions woven between.
            mm_order = []
            for dd in range(RT + 6 * (NCB - 1)):
                for cb in range(NCB):
                    rt = dd - 6 * cb
                    if 0 <= rt < RT:
                        mm_order.append((cb, rt))
            mm_i = 0

            def emit_mms(n):
                nonlocal mm_i
                emitted = 0
                while emitted < n and mm_i < len(mm_order):
                    cb, rt = mm_order[mm_i]
                    if xqT_tiles[rt] is None or cs_tiles[rt] is None:
                        break  # inputs not emitted yet; retry next call
                    emit_mm(cb, rt, mm_i)
                    mm_i += 1
                    emitted += 1

            # dummy AllReduce first: if the ~70us collective cost is one-time
            # ring setup, this absorbs it off the critical path
            nc.gpsimd.collective_compute(
                "AllReduce", ALU.add,
                replica_groups=[list(range(N_CORES))],
                ins=[ccw_in[:]], outs=[ccw_out[:]])

            wsum1 = emit_wscale_head()
            make_identity(nc, ident[:])

            xg_t = [None] * RT
            xg_t[0] = emit_x_front(0)
            xg_t[1] = emit_x_front(1)
            emit_stage(0, 0)
            emit_stage(0, 1)
            emit_x_back(0, xg_t[0])
            xg_t[2] = emit_x_front(2)
            emit_x_back(1, xg_t[1])
            xg_t[3] = emit_x_front(3)
            emit_x_back(2, xg_t[2])

            emit_wscale_tail(wsum1)
            for rt in range(4):
                emit_x_cs(rt)

            for j in range(4):
                emit_tern(0, j)
            xg_t[4] = emit_x_front(4)
            emit_x_cs(4)
            emit_x_back(3, xg_t[3])
            for j in range(4, KT):
                emit_tern(0, j)

            emit_stage(1, 0)
            xg_t[5] = emit_x_front(5)
            emit_x_cs(5)
            emit_x_back(4, xg_t[4])
            emit_mms(3)                      # (0,0) (0,1) (0,2)

            emit_stage(1, 1)
            xg_t[6] = emit_x_front(6)
            emit_x_cs(6)
            emit_x_back(5, xg_t[5])
            for j in range(KT):
                emit_tern(1, j)
            emit_mms(3)                      # (0,3) (0,4) (0,5)

            emit_stage(2, 0)
            xg_t[7] = emit_x_front(7)
            emit_x_cs(7)
            emit_x_back(6, xg_t[6])
            emit_mms(4)                      # d6,d7: (0,6) (1,0) (0,7) (1,1)

            emit_stage(2, 1)
            xg_t[8] = emit_x_front(8)
            emit_x_cs(8)
            emit_x_back(7, xg_t[7])
            for j in range(KT):
                emit_tern(2, j)
            emit_mms(4)                      # d8,d9

            xg_t[9] = emit_x_front(9)
            emit_x_cs(9)
            emit_x_back(8, xg_t[8])
            emit_stage(3, 0)
            emit_mms(4)                      # d10,d11

            xg_t[10] = emit_x_front(10)
            emit_x_cs(10)
            emit_x_back(9, xg_t[9])
            emit_stage(3, 1)
            for j in range(KT):
                emit_tern(3, j)
            emit_mms(6)                      # d12,d13

            for rt in range(11, 16):
                xg_t[rt] = emit_x_front(rt)
                emit_x_cs(rt)
                emit_x_back(rt - 1, xg_t[rt - 1])
                emit_mms(6)
            emit_x_back(15, xg_t[15])
            emit_mms(len(mm_order))          # the rest

    nc.compile()
    return nc


def _host_weight_layout(weight: np.ndarray) -> np.ndarray:
    """[N,K] weight -> wTs [K,N] where row ((cb*2+kh)*128+p), col (jj*1024+n)
    holds weight[cb*1024+n, (kh*4+jj)*128+p]. Stage block (cb,kh) is then a
    contiguous 128-row slice in [k-partition, (j, n)] layout."""
    wT = np.ascontiguousarray(weight.T)                    # [K, N]
    B = wT.reshape(2, 4, 128, NCB, CBW)                    # kh, jj, p, cb, n
    C = B.transpose(3, 0, 2, 1, 4)                         # cb, kh, p, jj, n
    return np.ascontiguousarray(C).reshape(K, N)


def _ensure_ntff_hook():
    """Make trace=True work: bass_utils imports antenv.axon_hooks, which is
    not present in this image. Shim it and install the ctypes-based NTFF
    profiling hook against libaxon_pjrt.so (same recipe as trn_boot)."""
    import sys
    import types
    try:
        import antenv.axon_hooks  # noqa: F401
        return
    except ImportError:
        pass
    mod = types.ModuleType("antenv.axon_hooks")
    mod._hook = None
    mod.set_axon_ntff_profile_hook = lambda h: setattr(mod, "_hook", h)
    mod.get_axon_ntff_profile_hook = lambda: mod._hook
    sys.modules["antenv.axon_hooks"] = mod
    import antenv
    antenv.axon_hooks = mod
    try:
        from trn_agent_boot.trn_boot import _ntff_profile_via_ctypes
        hook = _ntff_profile_via_ctypes("/opt/axon/libaxon_pjrt.so")
        if hook is not None:
            mod._hook = hook
    except Exception as e:  # degrade to no-trace
        print(f"ntff hook install failed: {e}")
    # no S3 in this sandbox; keep artifacts local
    import concourse.bass_utils as bu
    bu.upload_artifacts = lambda tmpdir: f"local://{tmpdir}"


_NC_CACHE = {}


def kernel(x: np.ndarray, weight: np.ndarray, norm_weight: np.ndarray) -> np.ndarray:
    x = np.ascontiguousarray(x, dtype=np.float32)
    weight = np.ascontiguousarray(weight, dtype=np.float32)
    norm_weight = np.ascontiguousarray(norm_weight, dtype=np.float32)

    B, S, Kin = x.shape
    xf = x.reshape(-1, Kin)
    g_is_ones = bool(np.all(norm_weight == 1.0))

    key = (g_is_ones,)
    if key not in _NC_CACHE:
        _NC_CACHE[key] = build_nc(g_is_ones)
    nc = _NC_CACHE[key]

    wTs = _host_weight_layout(weight)
    in_maps = []
    for i in range(N_CORES):
        m = {"x": xf[i * R:(i + 1) * R], "wTs": wTs,
             "wsh": wTs[i * 128:(i + 1) * 128]}
        if not g_is_ones:
            m["g"] = norm_weight.reshape(1, Kin)
        in_maps.append(m)

    trace = bool(int(os.environ.get("BITLIN_TRACE", "0")))
    if trace:
        _ensure_ntff_hook()
    res = run_bass_kernel_spmd(
        nc, in_maps, core_ids=list(range(N_CORES)), trace=trace,
    )
    if trace:
        kernel.last_results = res
    out = np.concatenate(
        [np.asarray(r["out"]).astype(np.float32) for r in res.results], axis=0)
    return out.reshape(B, S, weight.shape[0])
